# revision 32
# baseline (speedup 1.0000x reference)
"""DCGRU classifier kernel for Trainium2 (8 NeuronCores, batch-data-parallel).

v2: bf16 matmuls (4x PE throughput vs fp32), minimal elementwise op count,
h2 streamed to DRAM per step (host does the seq_length select + fc tail).

Layout (per core, B_loc=4 batch items, BN = 4*128 = 512):
  - Activations FEATURE-major: (features, batch*node).
  - gconv reordered as z@(W0-W2) + S@(z@W1) + (2S^2)@(z@W2).
    Projections q = z@W via matmul(lhsT=z_b, rhs=W) -> node-major psum;
    gate q1/q2 produced by ONE free=256 matmul per batch item.
    Diffusions [A@q]^T via matmul(lhsT=q_b, rhs=A^T) -> feature-major psum.
  - Z tiles: Z0 = [h1(0:64); x(64:80)], Z1 = [h1(0:64); h2(64:128)].
    All tensor_tensor operands live at base partition 0 (HW requires all
    TT operands to share a start partition); h2 therefore has a base-0
    master tile that is copied into Z1 rows 64:128 each step.
  - Two-stage software pipeline: iteration t emits L0(t) then L1(t-1) so
    both layers' matmuls interleave on the PE without stalls.
"""

import sys

import numpy as np
import ml_dtypes

sys.path.insert(0, "/opt/trn_rl_repo")

import concourse.bass as bass
import concourse.bacc as bacc
import concourse.mybir as mybir
from concourse.bass_utils import run_bass_kernel_spmd
from concourse.tile import TileContext

B, T, N, DIN, U, C = 32, 256, 128, 16, 64, 4
NCORES = 8
BL = B // NCORES  # 4 batch items per core
BN = BL * N  # 512
F32 = mybir.dt.float32
BF16 = mybir.dt.bfloat16
NPBF16 = ml_dtypes.bfloat16

SIG = mybir.ActivationFunctionType.Sigmoid
TANH = mybir.ActivationFunctionType.Tanh


# bf16 weight blob segments: (row_count, col_offset, col_count)
def _blob_layout():
    lay = {}
    col = 0

    def seg(key, rows, cols, row0=0):
        nonlocal col
        lay[key] = (rows, col, cols, row0)
        col += cols

    seg("S_T", N, N)
    seg("S2_T", N, N)
    for l, D in ((0, DIN + U), (1, 2 * U)):
        seg((l, "g12"), D, 4 * U)   # [W1 | W2] gate
        seg((l, "g0"), D, 2 * U)    # W0 - W2 gate
        seg((l, "c12"), D, 2 * U)   # [W1 | W2] cand
        seg((l, "c0"), D, U)        # W0 - W2 cand
    # layer1 cand weights split at the z-row boundary, rebased to row 0 so
    # split-operand matmuls (lhsT=rh2 at base 0) are legal
    seg((1, "c12r"), U, 2 * U)
    seg((1, "c0r"), U, U)
    return lay, col


_BLOB_LAYOUT, BLOB_COLS = _blob_layout()

_NC_CACHE = {}

import os
_BISECT = os.environ.get("KBISECT", "")


def _build_nc(t_steps: int):
    nc = bacc.Bacc("TRN2")

    xT_e = nc.declare_dram_parameter("xT", [t_steps, DIN, BN], BF16, isOutput=False)
    blob_e = nc.declare_dram_parameter("blob", [N, BLOB_COLS], BF16, isOutput=False)
    bias_e = nc.declare_dram_parameter("biases", [N, 4], F32, isOutput=False)
    h2_e = nc.declare_dram_parameter("h2hist", [t_steps, U, BN], BF16, isOutput=True)

    D0 = DIN + U  # 80

    with TileContext(nc) as tc:
        with (
            tc.tile_pool(name="singles", bufs=1) as singles,
            tc.tile_pool(name="z0b", bufs=3) as z0b_pool,
            tc.tile_pool(name="z0c", bufs=3) as z0c_pool,
            tc.tile_pool(name="z1b", bufs=3) as z1b_pool,
            tc.tile_pool(name="h2", bufs=3) as h2_pool,
            tc.tile_pool(name="rh2", bufs=2) as rh2_pool,
            tc.tile_pool(name="qg", bufs=4) as qg_pool,
            tc.tile_pool(name="qc", bufs=2) as qc_pool,
            tc.tile_pool(name="ru", bufs=2) as ru_pool,
            tc.tile_pool(name="u0", bufs=2) as u0_pool,
            tc.tile_pool(name="um", bufs=2) as um_pool,
            tc.tile_pool(name="uh", bufs=2) as uh_pool,
            tc.tile_pool(name="cact", bufs=2) as cc_pool,
            tc.tile_pool(name="mm", bufs=2) as m_pool,
            tc.tile_pool(name="pq", bufs=4, space="PSUM") as pq_pool,
            tc.tile_pool(name="pqc", bufs=2, space="PSUM") as pqc_pool,
            tc.tile_pool(name="pv", bufs=2, space="PSUM") as pv_pool,
        ):
            blob = singles.tile([N, BLOB_COLS], BF16)
            nc.sync.dma_start(out=blob, in_=blob_e[:, :])

            def wv(key):
                rows, c0, cols, row0 = _BLOB_LAYOUT[key]
                return blob[row0 : row0 + rows, c0 : c0 + cols]

            st = wv("S_T")
            s2t = wv("S2_T")
            w = {k: wv(k) for k in _BLOB_LAYOUT if isinstance(k, tuple)}
            biases = singles.tile([N, 4], F32)
            nc.sync.dma_start(out=biases, in_=bias_e[:, :])
            bias_ru = {0: biases[:, 0:1], 1: biases[:, 2:3]}
            bias_c = {0: biases[0:U, 1:2], 1: biases[0:U, 3:4]}

            # ---- initial state tiles ----
            z0b = z0b_pool.tile([D0, BN], BF16, tag="z0b")
            nc.vector.memset(z0b[0:U, :], 0.0)  # h1 = 0
            nc.sync.dma_start(out=z0b[U:D0, :], in_=xT_e[0])
            z0c = z0c_pool.tile([D0, BN], BF16, tag="z0c")
            nc.sync.dma_start(out=z0c[U:D0, :], in_=xT_e[0])
            z1b0 = z1b_pool.tile([2 * U, BN], BF16, tag="z1b")
            nc.vector.memset(z1b0[U : 2 * U, :], 0.0)  # h2 = 0
            h2m = h2_pool.tile([U, BN], BF16, tag="h2")
            nc.gpsimd.memset(h2m, 0.0)

            state = {"z0b": z0b, "z0c": z0c, "h2": h2m}

            # ---------- per-layer phase pieces (s = in-flight dict) ----------
            def ph_gate_proj(l, z):
                """q1 = z@W1, q2 = z@W2 (node-major psum, per-b)."""
                pq1 = pq_pool.tile([N, BL, 2 * U], F32, tag="pq")
                pq2 = pq_pool.tile([N, BL, 2 * U], F32, tag="pq")
                for b in range(BL):
                    zb = z[:, b * N : (b + 1) * N]
                    nc.tensor.matmul(pq1[:, b, :], lhsT=zb,
                                     rhs=w[l, "g12"][:, 0 : 2 * U],
                                     start=True, stop=True)
                    nc.tensor.matmul(pq2[:, b, :], lhsT=zb,
                                     rhs=w[l, "g12"][:, 2 * U : 4 * U],
                                     start=True, stop=True)
                return pq1, pq2

            def ph_gate_cast(s):
                # parallel casts: q1 on vector, q2 on scalar
                q1 = qg_pool.tile([N, BL, 2 * U], BF16, tag="qg")
                q2 = qg_pool.tile([N, BL, 2 * U], BF16, tag="qg")
                nc.vector.tensor_copy(q1, s["pq1"])
                nc.scalar.copy(q2, s["pq2"])
                s["q1"], s["q2"] = q1, q2

            def ph_gate_val(l, s, z):
                pval = pv_pool.tile([2 * U, BN], F32, tag="pv")
                nc.tensor.matmul(pval, lhsT=w[l, "g0"], rhs=z,
                                 start=True, stop=False)
                for b in range(BL):
                    blk = pval[:, b * N : (b + 1) * N]
                    nc.tensor.matmul(blk, lhsT=s["q1"][:, b, :], rhs=st,
                                     start=False, stop=False,
                                     skip_group_check=True)
                    nc.tensor.matmul(blk, lhsT=s["q2"][:, b, :], rhs=s2t,
                                     start=False, stop=(b == BL - 1),
                                     skip_group_check=True)
                s["pval"] = pval

            def ph_gate_act(l, s, h_prev):
                # merged r|u sigmoid (128 partitions); u realigned to base 0
                # off-chain; um = 1-u and uh = u*h also off the r-chain
                ru = ru_pool.tile([2 * U, BN], BF16, tag="ru")
                nc.scalar.activation(ru, s["pval"], SIG, bias=bias_ru[l])
                u0 = u0_pool.tile([U, BN], BF16, tag="u0")
                nc.vector.tensor_copy(u0, ru[U : 2 * U, :])
                um = um_pool.tile([U, BN], BF16, tag="um")
                nc.vector.tensor_scalar(um, u0, 1.0, -1.0,
                                        mybir.AluOpType.subtract,
                                        mybir.AluOpType.mult)
                uh = uh_pool.tile([U, BN], BF16, tag="uh")
                nc.gpsimd.tensor_mul(uh, u0, h_prev)
                s["ru"], s["um"], s["uh"] = ru, um, uh

            def ph_cand_l0(s):
                z0b, z0c = state["z0b"], state["z0c"]
                nc.vector.tensor_mul(z0c[0:U, :], s["ru"][0:U, :], z0b[0:U, :])
                pqc = pqc_pool.tile([N, BL, 2 * U], F32, tag="pqc")
                for b in range(BL):
                    nc.tensor.matmul(pqc[:, b, :],
                                     lhsT=z0c[:, b * N : (b + 1) * N],
                                     rhs=w[0, "c12"], start=True, stop=True)
                s["pqc"] = pqc

            def ph_cand_l1(s, z1b):
                rh2 = rh2_pool.tile([U, BN], BF16, tag="rh2")
                nc.vector.tensor_mul(rh2, s["ru"][0:U, :], state["h2"])
                pqc = pqc_pool.tile([N, BL, 2 * U], F32, tag="pqc")
                for b in range(BL):
                    sl = slice(b * N, (b + 1) * N)
                    nc.tensor.matmul(pqc[:, b, :], lhsT=z1b[0:U, sl],
                                     rhs=w[1, "c12"][0:U, :],
                                     start=True, stop=False)
                    nc.tensor.matmul(pqc[:, b, :], lhsT=rh2[:, sl],
                                     rhs=w[1, "c12r"],
                                     start=False, stop=True,
                                     skip_group_check=True)
                s["pqc"], s["rh2"] = pqc, rh2

            def ph_qc_cast(s):
                qc = qc_pool.tile([N, BL, 2 * U], BF16, tag="qc")
                nc.scalar.copy(qc, s["pqc"])
                s["qc"] = qc

            def ph_cand_val_l0(s):
                z0c = state["z0c"]
                pc = pv_pool.tile([U, BN], F32, tag="pv")
                nc.tensor.matmul(pc, lhsT=w[0, "c0"], rhs=z0c,
                                 start=True, stop=False)
                for b in range(BL):
                    blk = pc[:, b * N : (b + 1) * N]
                    nc.tensor.matmul(blk, lhsT=s["qc"][:, b, 0:U], rhs=st,
                                     start=False, stop=False,
                                     skip_group_check=True)
                    nc.tensor.matmul(blk, lhsT=s["qc"][:, b, U : 2 * U],
                                     rhs=s2t,
                                     start=False, stop=(b == BL - 1),
                                     skip_group_check=True)
                s["pc"] = pc

            def ph_cand_val_l1(s, z1b):
                pc = pv_pool.tile([U, BN], F32, tag="pv")
                nc.tensor.matmul(pc, lhsT=w[1, "c0"][0:U, :], rhs=z1b[0:U, :],
                                 start=True, stop=False)
                nc.tensor.matmul(pc, lhsT=w[1, "c0r"], rhs=s["rh2"],
                                 start=False, stop=False, skip_group_check=True)
                for b in range(BL):
                    blk = pc[:, b * N : (b + 1) * N]
                    nc.tensor.matmul(blk, lhsT=s["qc"][:, b, 0:U], rhs=st,
                                     start=False, stop=False,
                                     skip_group_check=True)
                    nc.tensor.matmul(blk, lhsT=s["qc"][:, b, U : 2 * U],
                                     rhs=s2t,
                                     start=False, stop=(b == BL - 1),
                                     skip_group_check=True)
                s["pc"] = pc

            def ph_c_act(l, s):
                cc = cc_pool.tile([U, BN], BF16, tag="cc")
                nc.scalar.activation(cc, s["pc"], TANH, bias=bias_c[l])
                s["cc"] = cc

            def ph_update(l, s, t, z1b_out):
                # h' = u*h + (1-u)*c = uh + um*c ; uh/um precomputed off-chain
                m = m_pool.tile([U, BN], BF16, tag="m")
                nc.vector.tensor_mul(m, s["um"], s["cc"])
                if l == 0:
                    nc.vector.tensor_add(z1b_out[0:U, :], s["uh"], m)
                    if t + 1 < t_steps:
                        z0b_n = z0b_pool.tile([D0, BN], BF16, tag="z0b")
                        nc.vector.tensor_copy(z0b_n[0:U, :], z1b_out[0:U, :])
                        nc.sync.dma_start(out=z0b_n[U:D0, :], in_=xT_e[t + 1])
                        z0c_n = z0c_pool.tile([D0, BN], BF16, tag="z0c")
                        nc.sync.dma_start(out=z0c_n[U:D0, :], in_=xT_e[t + 1])
                        state["z0b"], state["z0c"] = z0b_n, z0c_n
                else:
                    h2n = h2_pool.tile([U, BN], BF16, tag="h2")
                    nc.vector.tensor_add(h2n, s["uh"], m)
                    nc.vector.tensor_copy(z1b_out[U : 2 * U, :], h2n)
                    nc.sync.dma_start(out=h2_e[t], in_=h2n)
                    state["h2"] = h2n

            # ---------- interleaved two-layer pipeline ----------
            # iteration t: L0 computes step t, L1 computes step t-1.
            # Phases interleave so every engine alternates between layers.
            z1b_cur = z1b0
            for t in range(t_steps):
                z1b_prev = z1b_cur
                if t > 0:
                    z1b_cur = z1b_pool.tile([2 * U, BN], BF16, tag="z1b")
                sa = {}
                sb = {}
                z0b_t = state["z0b"]
                sa["pq1"], sa["pq2"] = ph_gate_proj(0, z0b_t)
                if t > 0:
                    sb["pq1"], sb["pq2"] = ph_gate_proj(1, z1b_prev)
                ph_gate_cast(sa)
                if t > 0:
                    ph_gate_cast(sb)
                ph_gate_val(0, sa, z0b_t)
                if t > 0:
                    ph_gate_val(1, sb, z1b_prev)
                ph_gate_act(0, sa, z0b_t[0:U, :])
                ph_cand_l0(sa)
                if t > 0:
                    ph_gate_act(1, sb, state["h2"])
                    ph_cand_l1(sb, z1b_prev)
                ph_qc_cast(sa)
                if t > 0:
                    ph_qc_cast(sb)
                ph_cand_val_l0(sa)
                if t > 0:
                    ph_cand_val_l1(sb, z1b_prev)
                ph_c_act(0, sa)
                if t > 0:
                    ph_c_act(1, sb)
                ph_update(0, sa, t, z1b_cur)
                if t > 0:
                    ph_update(1, sb, t - 1, z1b_cur)
            # tail: L1 step t_steps-1
            z1b_prev = z1b_cur
            z1b_tail = z1b_pool.tile([2 * U, BN], BF16, tag="z1b")
            sb = {}
            sb["pq1"], sb["pq2"] = ph_gate_proj(1, z1b_prev)
            ph_gate_cast(sb)
            ph_gate_val(1, sb, z1b_prev)
            ph_gate_act(1, sb, state["h2"])
            ph_cand_l1(sb, z1b_prev)
            ph_qc_cast(sb)
            ph_cand_val_l1(sb, z1b_prev)
            ph_c_act(1, sb)
            ph_update(1, sb, t_steps - 1, z1b_tail)

    nc.compile()
    return nc


def _prep_shared(support, W0_gate, b0_gate, W0_cand, b0_cand,
                 W1_gate, b1_gate, W1_cand, b1_cand):
    f = np.float32
    S = np.asarray(support, f)
    seg = {
        "S_T": np.ascontiguousarray(S.T),
        "S2_T": np.ascontiguousarray((2.0 * (S @ S)).T),
    }
    biases = np.zeros((N, 4), f)
    for l, (Wg, Wc, bg, bc) in enumerate(
        ((W0_gate, W0_cand, b0_gate, b0_cand), (W1_gate, W1_cand, b1_gate, b1_cand))
    ):
        Wg = np.asarray(Wg, f)
        Wc = np.asarray(Wc, f)
        g = [Wg[m::3] for m in range(3)]
        c = [Wc[m::3] for m in range(3)]
        if l == 0:
            # device z-layout for layer0 is [h(64); x(16)]
            perm = np.concatenate([np.arange(DIN, DIN + U), np.arange(DIN)])
            g = [gm[perm] for gm in g]
            c = [cm[perm] for cm in c]
        seg[(l, "g12")] = np.concatenate([g[1], g[2]], axis=1)
        seg[(l, "g0")] = g[0] - g[2]
        seg[(l, "c12")] = np.concatenate([c[1], c[2]], axis=1)
        seg[(l, "c0")] = c[0] - c[2]
        if l == 1:
            seg[(1, "c12r")] = seg[(1, "c12")][U : 2 * U]
            seg[(1, "c0r")] = seg[(1, "c0")][U : 2 * U]
        biases[:, 2 * l] = np.asarray(bg, f).reshape(-1)  # [bgr; bgu]
        biases[0:U, 2 * l + 1] = np.asarray(bc, f).reshape(-1)
    blob = np.zeros((N, BLOB_COLS), NPBF16)
    for key, (rows, c0, cols, row0) in _BLOB_LAYOUT.items():
        a = seg[key]
        assert a.shape == (rows, cols), (key, a.shape, rows, cols)
        blob[row0 : row0 + rows, c0 : c0 + cols] = a.astype(NPBF16)
    return {"blob": blob, "biases": biases}


def run_cores(inputs, t_steps=T, trace=False):
    input_seq = np.asarray(inputs["input_seq"], np.float32)
    shared = _prep_shared(
        inputs["support"], inputs["W0_gate"], inputs["b0_gate"],
        inputs["W0_cand"], inputs["b0_cand"],
        inputs["W1_gate"], inputs["b1_gate"],
        inputs["W1_cand"], inputs["b1_cand"],
    )
    in_maps = []
    for k in range(NCORES):
        xs = input_seq[k * BL : (k + 1) * BL, :t_steps]  # (BL, t, N, DIN)
        xT = np.ascontiguousarray(
            np.transpose(xs, (1, 3, 0, 2)).reshape(t_steps, DIN, BN)
        ).astype(NPBF16)
        in_maps.append(dict(shared, xT=xT))
    if t_steps not in _NC_CACHE:
        _NC_CACHE[t_steps] = _build_nc(t_steps)
    nc = _NC_CACHE[t_steps]
    res = run_bass_kernel_spmd(nc, in_maps, list(range(NCORES)), trace=trace)
    return res


def finish_host(results, inputs, t_steps=T):
    """Host tail: pick h2 at t = seq_len-1, relu -> fc -> node max-pool."""
    W_fc = np.asarray(inputs["W_fc"], np.float32)
    b_fc = np.asarray(inputs["b_fc"], np.float32)
    seq_lengths = np.asarray(inputs["seq_lengths"]).astype(np.int64)
    out = np.empty((B, C), np.float32)
    for k in range(NCORES):
        h2 = np.asarray(results[k]["h2hist"], dtype=np.float32)  # (t, U, BN)
        for b in range(BL):
            t_idx = min(int(seq_lengths[k * BL + b]) - 1, t_steps - 1)
            blk = h2[t_idx, :, b * N : (b + 1) * N].T  # (N, U)
            logits = np.maximum(blk, 0.0) @ W_fc + b_fc  # (N, C)
            out[k * BL + b] = logits.max(axis=0)
    return out


def kernel(**inputs):
    res = run_cores(inputs, t_steps=T)
    return finish_host(res.results, inputs, t_steps=T)
